# revision 34
# baseline (speedup 1.0000x reference)
"""BoW extractor (VQ codebook softmax + border-cropped mean pool) on 8 Trainium2 cores.

Data-parallel over the batch dim: each of the 8 NeuronCores handles 16 images.
Per core, tokens are flattened to [3136, 768] (padded to 3200 = 25 tiles of 128)
and processed tile-by-tile:
  logits = 30 * (x @ emb.T) / ||x||   (fp32r matmuls, C contracted in 6 chunks)
  codes  = exp(logits) / row_sum      (ACT exp with per-partition scale + fused
                                       row-sums, DVE reciprocal + scale)
  bow    = L1norm(mean of kept codes) (PE matmul against a precomputed selector
                                       W, PSUM results DVE-accumulated in SBUF)
The first 4 tiles are processed codebook-block-outer so the PE consumes the
codebook in DMA-arrival order (keeps the PE dense at startup, which also
keeps the HAM clock-gate open). ACT sqrt for the token norms is batched per
5 tiles (one pipeline stage ahead) so the exp/sqrt table sets are not
reloaded per tile. Measured ~403-405us on core 0, rel err ~4.3e-4 vs the
fp32 reference (fp32r matmul rounding).
"""
import sys

sys.path.insert(0, "/opt/trn_rl_repo")

import numpy as np

N_CORES = 8
N_IMG = 128
C = 768
K = 4096
L = 196  # tokens per image after dropping CLS
IMG_PER_CORE = N_IMG // N_CORES  # 16
T_TOK = IMG_PER_CORE * L  # 3136
NT = 25  # 128-token tiles per core
T_PAD = NT * 128  # 3200
GRID = 14
SKIP = 2
INV_DELTA = 30.0  # 15.0 / 0.5
NORMALIZE_EPS = 1e-5
NB = 5  # tiles per norm (sqrt) batch
NPRE = 5  # token tiles prefetched ahead of the codebook load

PROFILE = False
LAST_EXEC_NS = None

_PROG = None
_HOST_CONST = None


def _build_program():
    import concourse.bacc as bacc
    import concourse.tile as tile
    from concourse import mybir

    f32 = mybir.dt.float32
    f32r = mybir.dt.float32r
    bf16 = mybir.dt.bfloat16
    AF = mybir.ActivationFunctionType
    AX = mybir.AxisListType

    nc = bacc.Bacc("TRN2", target_bir_lowering=False, debug=False,
                   num_devices=N_CORES)
    xT_d = nc.dram_tensor("xT", [NT, 128, 6, 128], f32r, kind="ExternalInput")
    xn_d = nc.dram_tensor("xnat", [NT, 128, C], f32, kind="ExternalInput")
    emb_d = nc.dram_tensor("embT", [8, 128, 6, 512], f32r,
                           kind="ExternalInput")
    w_d = nc.dram_tensor("W", [128, NT, 16], f32r, kind="ExternalInput")
    codes_d = nc.dram_tensor("codes", [NT, 128, 8, 512], f32r,
                             kind="ExternalOutput")
    bow_d = nc.dram_tensor("bow", [16, 8, 512], f32, kind="ExternalOutput")

    with tile.TileContext(nc) as tc:
        with (
            tc.tile_pool(name="const", bufs=1) as constp,
            tc.tile_pool(name="xT", bufs=NPRE, space="SBUF") as xTp,
            tc.tile_pool(name="xn", bufs=2) as xnp_,
            tc.tile_pool(name="expp", bufs=4) as expp,
            tc.tile_pool(name="small", bufs=4) as smallp,
            tc.tile_pool(name="ps", bufs=6, space="PSUM") as psp,
            tc.tile_pool(name="psbow", bufs=2, space="PSUM") as psbowp,
        ):
            # --- PE warmup: dependency-free dummy matmuls run during the
            # initial DMAs so the HAM clock-gate opens before real work.
            warm_sb = constp.tile([128, 128], bf16)
            warm_ps = psp.tile([128, 512], f32, name="warm_ps", tag="ps")
            nc.vector.memset(warm_sb[:], 0.0)
            for i in range(60):
                nc.tensor.matmul(warm_ps[:, :128], warm_sb[:], warm_sb[:],
                                 start=True, stop=True)

            # --- prefetch the first NPRE token tiles ahead of the codebook
            # (xn batch 0 is sandwiched after the first two xT tiles: the
            # softmax scales gate PSUM evacuation, so the norm inputs must
            # land before the startup blocks fill all PSUM banks)
            pre_xT = []

            def prefetch_xT(t):
                a = xTp.tile([128, 6, 128], f32r, name=f"xTpre{t}",
                             tag="xT_t")
                nc.sync.dma_start(out=a[:], in_=xT_d.ap()[t])
                pre_xT.append(a)

            for t in range(2):
                prefetch_xT(t)

            bow_acc = constp.tile([16, 8, 512], f32)
            nc.gpsimd.memset(bow_acc[:], 0.0)

            # --- norm pipeline, one NB-tile batch ahead of the main loop:
            # squares accumulate ||x||^2, one batched ACT sqrt per NB tiles
            # (2 table-set reloads per batch instead of per tile)
            n2_all = constp.tile([128, NT], f32)
            sinv_all = constp.tile([128, NT], f32)

            def norm_batch(g):
                lo, hi = NB * g, min(NB * (g + 1), NT)
                for u in range(lo, hi):
                    xn_u = xnp_.tile([128, C], f32, name=f"xn{u}", tag="xn")
                    nc.sync.dma_start(out=xn_u[:], in_=xn_d.ap()[u])
                    nc.vector.scalar_tensor_tensor(
                        xn_u[:], xn_u[:], 1.0, xn_u[:],
                        mybir.AluOpType.mult, mybir.AluOpType.mult,
                        accum_out=n2_all[:, u:u + 1])
                nc.vector.tensor_scalar_max(n2_all[:, lo:hi],
                                            n2_all[:, lo:hi],
                                            NORMALIZE_EPS * NORMALIZE_EPS)
                nrm = smallp.tile([128, NB], f32, name=f"nrm{g}", tag="nrm")
                nc.scalar.activation(nrm[:, :hi - lo], n2_all[:, lo:hi],
                                     AF.Sqrt,
                                     scale=1.0 / (INV_DELTA * INV_DELTA))
                nc.vector.reciprocal(sinv_all[:, lo:hi], nrm[:, :hi - lo])

            norm_batch(0)
            for t in range(2, NPRE):
                prefetch_xT(t)

            # --- codebook load split across both HWDGE rings, interleaved
            # so blocks land roughly in consumption order
            emb_sb = constp.tile([128, 8, 6, 512], f32r)
            for j in range(8):
                # ACT ring serves the first-consumed blocks; the SP ring
                # (busy with x prefetches first) serves the later ones
                eng = nc.scalar if j < 3 else nc.sync
                eng.dma_start(out=emb_sb[:, j], in_=emb_d.ap()[j])
            w_sb = constp.tile([128, NT, 16], f32r)
            nc.scalar.dma_start(out=w_sb[:], in_=w_d.ap())

            def get_xT(t):
                if t < NPRE:
                    return pre_xT[t]
                xT_t = xTp.tile([128, 6, 128], f32r, name=f"xT{t}",
                                tag="xT_t")
                nc.sync.dma_start(out=xT_t[:], in_=xT_d.ap()[t])
                return xT_t

            def new_exp_sums(t):
                exp_t = expp.tile([128, 8, 512], f32r, name=f"exp{t}",
                                  tag="exp_t")
                sums = smallp.tile([128, 8], f32, name=f"sums{t}",
                                   tag="sums")
                return exp_t, sums

            def do_block(t, j, xT_t, exp_t, sums):
                ps = psp.tile([128, 512], f32, name=f"ps{t}_{j}", tag="ps")
                for c in range(6):
                    nc.tensor.matmul(ps[:], xT_t[:, c, :],
                                     emb_sb[:, j, c, :],
                                     start=(c == 0), stop=(c == 5))
                nc.scalar.activation(exp_t[:, j, :], ps[:], AF.Exp,
                                     scale=sinv_all[:, t:t + 1],
                                     accum_out=sums[:, j:j + 1])

            def do_post(t, exp_t, sums):
                # tile NT-1 holds only 64 real tokens (rest is padding)
                rows = 64 if t == NT - 1 else 128
                denom = smallp.tile([128, 1], f32, name=f"dn{t}",
                                    tag="denom")
                nc.vector.reduce_sum(denom[:rows], sums[:rows], axis=AX.X)
                r = smallp.tile([128, 1], f32, name=f"r{t}", tag="r")
                nc.vector.reciprocal(r[:rows], denom[:rows])
                for j in range(8):
                    nc.vector.tensor_scalar_mul(exp_t[:rows, j, :],
                                                exp_t[:rows, j, :], r[:rows])
                    bow_tmp = psbowp.tile([16, 512], f32,
                                          name=f"bt{t}_{j}", tag="bt")
                    nc.tensor.matmul(bow_tmp[:], w_sb[:rows, t, :],
                                     exp_t[:rows, j, :], start=True,
                                     stop=True)
                    nc.vector.tensor_add(bow_acc[:, j, :], bow_acc[:, j, :],
                                         bow_tmp[:])
                    if j % 2 == 1:
                        deng = nc.sync if t == NT - 1 else nc.gpsimd
                        deng.dma_start(
                            out=codes_d.ap()[t][:rows, j - 1:j + 1, :],
                            in_=exp_t[:rows, j - 1:j + 1, :])

            # --- startup: first tiles block-outer (follows emb DMA arrival)
            NSTART = 4
            start_bufs = [new_exp_sums(t) for t in range(NSTART)]
            for j in range(8):
                for t in range(NSTART):
                    do_block(t, j, pre_xT[t], *start_bufs[t])
                if 2 <= j <= 6:
                    # filler matmuls gated on block j keep the PE (and the
                    # HAM clock) busy while block j+1's DMA is in flight;
                    # they use the bow PSUM slots, idle during startup
                    for k in range(10):
                        dmy = psbowp.tile([16, 512], f32,
                                          name=f"wd{j}_{k}", tag="bt")
                        nc.tensor.matmul(dmy[:], emb_sb[:, j, 0, 0:16],
                                         emb_sb[:, j, 0, :],
                                         start=True, stop=True)
            for t in range(NSTART):
                do_post(t, *start_bufs[t])

            # --- steady state
            batches_done = 1
            last_batch = (NT - 1) // NB
            for t in range(NSTART, NT):
                # keep the norm pipeline ~2 tiles ahead of consumption
                while batches_done <= min((t + 2) // NB, last_batch):
                    norm_batch(batches_done)
                    batches_done += 1
                xT_t = get_xT(t)
                exp_t, sums = new_exp_sums(t)
                for j in range(8):
                    do_block(t, j, xT_t, exp_t, sums)
                do_post(t, exp_t, sums)

            # --- L1-normalize bow per image (rows are images); the
            # reduce and the final scale are split DVE/ACT so the two
            # halves run concurrently in the kernel tail
            ssum_a = smallp.tile([16, 1], f32)
            nc.vector.reduce_sum(ssum_a[:], bow_acc[:, 0:4, :], axis=AX.XY)
            ssum_b = smallp.tile([16, 1], f32)
            nc.scalar.activation(bow_acc[:, 4:8, :], bow_acc[:, 4:8, :],
                                 AF.Identity, accum_out=ssum_b[:])
            ssum = smallp.tile([16, 1], f32)
            nc.vector.tensor_add(ssum[:], ssum_a[:], ssum_b[:])
            nc.vector.tensor_scalar_max(ssum[:], ssum[:], NORMALIZE_EPS)
            rimg = smallp.tile([16, 1], f32)
            nc.vector.reciprocal(rimg[:], ssum[:])
            nc.vector.tensor_scalar_mul(bow_acc[:, 0:4, :],
                                        bow_acc[:, 0:4, :], rimg[:])
            nc.scalar.activation(bow_acc[:, 4:8, :], bow_acc[:, 4:8, :],
                                 AF.Copy, scale=rimg[:])
            nc.gpsimd.dma_start(out=bow_d.ap(), in_=bow_acc[:])

    nc.compile()
    return nc


def _host_constants():
    global _HOST_CONST
    if _HOST_CONST is not None:
        return _HOST_CONST
    # kept-token mask on the 14x14 grid (drop SKIP-wide border)
    l_idx = np.arange(L)
    row, col = l_idx // GRID, l_idx % GRID
    kept = ((row >= SKIP) & (row < GRID - SKIP) &
            (col >= SKIP) & (col < GRID - SKIP))
    n_keep = int(kept.sum())  # 100
    w_full = np.zeros((T_PAD, IMG_PER_CORE), np.float32)
    for i in range(IMG_PER_CORE):
        w_full[i * L:(i + 1) * L, i] = kept / float(n_keep)
    w3 = np.ascontiguousarray(
        w_full.reshape(NT, 128, IMG_PER_CORE).transpose(1, 0, 2))
    _HOST_CONST = w3
    return _HOST_CONST


def _get_program():
    global _PROG
    if _PROG is None:
        _PROG = _build_program()
    return _PROG


def kernel(x, embedding):
    global LAST_EXEC_NS
    from concourse.bass_utils import run_bass_kernel_spmd

    x = np.asarray(x, dtype=np.float32)
    embedding = np.asarray(embedding, dtype=np.float32)
    nc = _get_program()
    w3 = _host_constants()

    embT = np.ascontiguousarray(
        embedding.T.reshape(6, 128, 8, 512).transpose(2, 1, 0, 3))

    in_maps = []
    for core in range(N_CORES):
        xc = x[core * IMG_PER_CORE:(core + 1) * IMG_PER_CORE, 1:, :]
        xp = np.zeros((T_PAD, C), np.float32)
        xp[:T_TOK] = xc.reshape(T_TOK, C)
        xp[T_TOK:, 0] = 1.0  # pad tokens: unit norm, zero pool weight
        in_maps.append({
            "xT": np.ascontiguousarray(
                xp.reshape(NT, 128, 6, 128).transpose(0, 3, 2, 1)),
            "xnat": xp.reshape(NT, 128, C),
            "embT": embT,
            "W": w3,
        })

    res = run_bass_kernel_spmd(nc, in_maps, core_ids=list(range(N_CORES)),
                               trace=PROFILE)
    LAST_EXEC_NS = res.exec_time_ns

    bow = np.empty((N_IMG, K), np.float32)
    codes = np.empty((N_IMG, L, K), np.float32)
    for core in range(N_CORES):
        sl = slice(core * IMG_PER_CORE, (core + 1) * IMG_PER_CORE)
        codes[sl] = (res.results[core]["codes"]
                     .reshape(T_PAD, K)[:T_TOK]
                     .reshape(IMG_PER_CORE, L, K))
        bow[sl] = res.results[core]["bow"].reshape(IMG_PER_CORE, K)
    return bow, codes


# revision 35
# speedup vs baseline: 1.0449x; 1.0449x over previous
"""BoW extractor (VQ codebook softmax + border-cropped mean pool) on 8 Trainium2 cores.

Data-parallel over the batch dim: each of the 8 NeuronCores handles 16 images.
Per core, tokens are flattened to [3136, 768] (padded to 3200 = 25 tiles of 128)
and processed tile-by-tile:
  logits = 30 * (x @ emb.T) / ||x||   (fp32r matmuls, C contracted in 6 chunks)
  codes  = exp(logits) / row_sum      (ACT exp with per-partition scale + fused
                                       row-sums, DVE reciprocal + scale)
  bow    = L1norm(mean of kept codes) (PE matmul against a precomputed selector
                                       W, PSUM results DVE-accumulated in SBUF)
The first 4 tiles are processed codebook-block-outer so the PE consumes the
codebook in DMA-arrival order (keeps the PE dense at startup, which also
keeps the HAM clock-gate open). ACT sqrt for the token norms is batched per
5 tiles (one pipeline stage ahead) so the exp/sqrt table sets are not
reloaded per tile. Measured ~403-405us on core 0, rel err ~4.3e-4 vs the
fp32 reference (fp32r matmul rounding).
"""
import sys

sys.path.insert(0, "/opt/trn_rl_repo")

import numpy as np

N_CORES = 8
N_IMG = 128
C = 768
K = 4096
L = 196  # tokens per image after dropping CLS
IMG_PER_CORE = N_IMG // N_CORES  # 16
T_TOK = IMG_PER_CORE * L  # 3136
NT = 25  # 128-token tiles per core
T_PAD = NT * 128  # 3200
GRID = 14
SKIP = 2
INV_DELTA = 30.0  # 15.0 / 0.5
NORMALIZE_EPS = 1e-5
NB = 5  # tiles per norm (sqrt) batch
NPRE = 5  # token tiles prefetched ahead of the codebook load

PROFILE = False
LAST_EXEC_NS = None

_PROG = None
_HOST_CONST = None


def _build_program():
    import concourse.bacc as bacc
    import concourse.tile as tile
    from concourse import mybir

    f32 = mybir.dt.float32
    f32r = mybir.dt.float32r
    bf16 = mybir.dt.bfloat16
    AF = mybir.ActivationFunctionType
    AX = mybir.AxisListType

    nc = bacc.Bacc("TRN2", target_bir_lowering=False, debug=False,
                   num_devices=N_CORES)
    xT_d = nc.dram_tensor("xT", [NT, 128, 6, 128], f32r, kind="ExternalInput")
    xn_d = nc.dram_tensor("xnat", [NT, 128, C], f32, kind="ExternalInput")
    emb_d = nc.dram_tensor("embT", [8, 128, 6, 512], f32r,
                           kind="ExternalInput")
    w_d = nc.dram_tensor("W", [128, NT, 16], f32r, kind="ExternalInput")
    codes_d = nc.dram_tensor("codes", [NT, 128, 8, 512], f32r,
                             kind="ExternalOutput")
    bow_d = nc.dram_tensor("bow", [16, 8, 512], f32, kind="ExternalOutput")

    with tile.TileContext(nc) as tc:
        with (
            tc.tile_pool(name="const", bufs=1) as constp,
            tc.tile_pool(name="xT", bufs=NPRE, space="SBUF") as xTp,
            tc.tile_pool(name="xn", bufs=2) as xnp_,
            tc.tile_pool(name="expp", bufs=4) as expp,
            tc.tile_pool(name="small", bufs=4) as smallp,
            tc.tile_pool(name="ps", bufs=6, space="PSUM") as psp,
            tc.tile_pool(name="psbow", bufs=2, space="PSUM") as psbowp,
        ):
            # --- PE warmup: dependency-free dummy matmuls run during the
            # initial DMAs so the HAM clock-gate opens before real work.
            warm_sb = constp.tile([128, 128], bf16)
            warm_ps = psp.tile([128, 512], f32, name="warm_ps", tag="ps")
            nc.vector.memset(warm_sb[:], 0.0)
            for i in range(60):
                nc.tensor.matmul(warm_ps[:, :128], warm_sb[:], warm_sb[:],
                                 start=True, stop=True)

            # --- prefetch the first NPRE token tiles ahead of the codebook
            # (xn batch 0 is sandwiched after the first two xT tiles: the
            # softmax scales gate PSUM evacuation, so the norm inputs must
            # land before the startup blocks fill all PSUM banks)
            pre_xT = []

            def prefetch_xT(t):
                a = xTp.tile([128, 6, 128], f32r, name=f"xTpre{t}",
                             tag="xT_t")
                nc.sync.dma_start(out=a[:], in_=xT_d.ap()[t])
                pre_xT.append(a)

            for t in range(2):
                prefetch_xT(t)

            bow_acc = constp.tile([16, 8, 512], f32)
            nc.gpsimd.memset(bow_acc[:], 0.0)

            # --- norm pipeline, one NB-tile batch ahead of the main loop:
            # squares accumulate ||x||^2, one batched ACT sqrt per NB tiles
            # (2 table-set reloads per batch instead of per tile)
            n2_all = constp.tile([128, NT], f32)
            sinv_all = constp.tile([128, NT], f32)

            def norm_batch(g):
                lo, hi = NB * g, min(NB * (g + 1), NT)
                for u in range(lo, hi):
                    xn_u = xnp_.tile([128, C], f32, name=f"xn{u}", tag="xn")
                    nc.sync.dma_start(out=xn_u[:], in_=xn_d.ap()[u])
                    nc.vector.scalar_tensor_tensor(
                        xn_u[:], xn_u[:], 1.0, xn_u[:],
                        mybir.AluOpType.mult, mybir.AluOpType.mult,
                        accum_out=n2_all[:, u:u + 1])
                nc.vector.tensor_scalar_max(n2_all[:, lo:hi],
                                            n2_all[:, lo:hi],
                                            NORMALIZE_EPS * NORMALIZE_EPS)
                nrm = smallp.tile([128, NB], f32, name=f"nrm{g}", tag="nrm")
                nc.scalar.activation(nrm[:, :hi - lo], n2_all[:, lo:hi],
                                     AF.Sqrt,
                                     scale=1.0 / (INV_DELTA * INV_DELTA))
                nc.vector.reciprocal(sinv_all[:, lo:hi], nrm[:, :hi - lo])

            norm_batch(0)
            for t in range(2, NPRE):
                prefetch_xT(t)

            # --- codebook load split across both HWDGE rings, interleaved
            # so blocks land roughly in consumption order
            emb_sb = constp.tile([128, 8, 6, 512], f32r)
            for j in range(8):
                # ACT ring serves the first-consumed blocks; the SP ring
                # (busy with x prefetches first) serves the later ones
                eng = nc.scalar if j < 3 else nc.sync
                eng.dma_start(out=emb_sb[:, j], in_=emb_d.ap()[j])
            w_sb = constp.tile([128, NT, 16], f32r)
            nc.scalar.dma_start(out=w_sb[:], in_=w_d.ap())

            def get_xT(t):
                if t < NPRE:
                    return pre_xT[t]
                xT_t = xTp.tile([128, 6, 128], f32r, name=f"xT{t}",
                                tag="xT_t")
                nc.sync.dma_start(out=xT_t[:], in_=xT_d.ap()[t])
                return xT_t

            def new_exp_sums(t):
                exp_t = expp.tile([128, 8, 512], f32r, name=f"exp{t}",
                                  tag="exp_t")
                sums = smallp.tile([128, 8], f32, name=f"sums{t}",
                                   tag="sums")
                return exp_t, sums

            def do_block(t, j, xT_t, exp_t, sums):
                ps = psp.tile([128, 512], f32, name=f"ps{t}_{j}", tag="ps")
                for c in range(6):
                    nc.tensor.matmul(ps[:], xT_t[:, c, :],
                                     emb_sb[:, j, c, :],
                                     start=(c == 0), stop=(c == 5))
                nc.scalar.activation(exp_t[:, j, :], ps[:], AF.Exp,
                                     scale=sinv_all[:, t:t + 1],
                                     accum_out=sums[:, j:j + 1])

            def do_post(t, exp_t, sums):
                # tile NT-1 holds only 64 real tokens (rest is padding)
                rows = 64 if t == NT - 1 else 128
                denom = smallp.tile([128, 1], f32, name=f"dn{t}",
                                    tag="denom")
                nc.vector.reduce_sum(denom[:rows], sums[:rows], axis=AX.X)
                r = smallp.tile([128, 1], f32, name=f"r{t}", tag="r")
                nc.vector.reciprocal(r[:rows], denom[:rows])
                for j in range(8):
                    nc.vector.tensor_scalar_mul(exp_t[:rows, j, :],
                                                exp_t[:rows, j, :], r[:rows])
                    bow_tmp = psbowp.tile([16, 512], f32,
                                          name=f"bt{t}_{j}", tag="bt")
                    nc.tensor.matmul(bow_tmp[:], w_sb[:rows, t, :],
                                     exp_t[:rows, j, :], start=True,
                                     stop=True)
                    nc.vector.tensor_add(bow_acc[:, j, :], bow_acc[:, j, :],
                                         bow_tmp[:])
                    if j % 2 == 1:
                        deng = nc.sync if t == NT - 1 else nc.gpsimd
                        deng.dma_start(
                            out=codes_d.ap()[t][:rows, j - 1:j + 1, :],
                            in_=exp_t[:rows, j - 1:j + 1, :])

            # --- startup: tiles 0-2 block-outer (follows emb DMA arrival)
            NSTART = 4
            start_bufs = [new_exp_sums(t) for t in range(NSTART)]
            for j in range(8):
                for t in range(NSTART):
                    do_block(t, j, pre_xT[t], *start_bufs[t])
            for t in range(NSTART):
                do_post(t, *start_bufs[t])

            # --- steady state
            batches_done = 1
            last_batch = (NT - 1) // NB
            for t in range(NSTART, NT):
                # keep the norm pipeline ~2 tiles ahead of consumption
                while batches_done <= min((t + 2) // NB, last_batch):
                    norm_batch(batches_done)
                    batches_done += 1
                xT_t = get_xT(t)
                exp_t, sums = new_exp_sums(t)
                for j in range(8):
                    do_block(t, j, xT_t, exp_t, sums)
                do_post(t, exp_t, sums)

            # --- L1-normalize bow per image (rows are images); the
            # reduce and the final scale are split DVE/ACT so the two
            # halves run concurrently in the kernel tail
            ssum_a = smallp.tile([16, 1], f32)
            nc.vector.reduce_sum(ssum_a[:], bow_acc[:, 0:4, :], axis=AX.XY)
            ssum_b = smallp.tile([16, 1], f32)
            nc.scalar.activation(bow_acc[:, 4:8, :], bow_acc[:, 4:8, :],
                                 AF.Identity, accum_out=ssum_b[:])
            ssum = smallp.tile([16, 1], f32)
            nc.vector.tensor_add(ssum[:], ssum_a[:], ssum_b[:])
            nc.vector.tensor_scalar_max(ssum[:], ssum[:], NORMALIZE_EPS)
            rimg = smallp.tile([16, 1], f32)
            nc.vector.reciprocal(rimg[:], ssum[:])
            nc.vector.tensor_scalar_mul(bow_acc[:, 0:4, :],
                                        bow_acc[:, 0:4, :], rimg[:])
            nc.scalar.activation(bow_acc[:, 4:8, :], bow_acc[:, 4:8, :],
                                 AF.Copy, scale=rimg[:])
            nc.gpsimd.dma_start(out=bow_d.ap(), in_=bow_acc[:])

    nc.compile()
    return nc


def _host_constants():
    global _HOST_CONST
    if _HOST_CONST is not None:
        return _HOST_CONST
    # kept-token mask on the 14x14 grid (drop SKIP-wide border)
    l_idx = np.arange(L)
    row, col = l_idx // GRID, l_idx % GRID
    kept = ((row >= SKIP) & (row < GRID - SKIP) &
            (col >= SKIP) & (col < GRID - SKIP))
    n_keep = int(kept.sum())  # 100
    w_full = np.zeros((T_PAD, IMG_PER_CORE), np.float32)
    for i in range(IMG_PER_CORE):
        w_full[i * L:(i + 1) * L, i] = kept / float(n_keep)
    w3 = np.ascontiguousarray(
        w_full.reshape(NT, 128, IMG_PER_CORE).transpose(1, 0, 2))
    _HOST_CONST = w3
    return _HOST_CONST


def _get_program():
    global _PROG
    if _PROG is None:
        _PROG = _build_program()
    return _PROG


def kernel(x, embedding):
    global LAST_EXEC_NS
    from concourse.bass_utils import run_bass_kernel_spmd

    x = np.asarray(x, dtype=np.float32)
    embedding = np.asarray(embedding, dtype=np.float32)
    nc = _get_program()
    w3 = _host_constants()

    embT = np.ascontiguousarray(
        embedding.T.reshape(6, 128, 8, 512).transpose(2, 1, 0, 3))

    in_maps = []
    for core in range(N_CORES):
        xc = x[core * IMG_PER_CORE:(core + 1) * IMG_PER_CORE, 1:, :]
        xp = np.zeros((T_PAD, C), np.float32)
        xp[:T_TOK] = xc.reshape(T_TOK, C)
        xp[T_TOK:, 0] = 1.0  # pad tokens: unit norm, zero pool weight
        in_maps.append({
            "xT": np.ascontiguousarray(
                xp.reshape(NT, 128, 6, 128).transpose(0, 3, 2, 1)),
            "xnat": xp.reshape(NT, 128, C),
            "embT": embT,
            "W": w3,
        })

    res = run_bass_kernel_spmd(nc, in_maps, core_ids=list(range(N_CORES)),
                               trace=PROFILE)
    LAST_EXEC_NS = res.exec_time_ns

    bow = np.empty((N_IMG, K), np.float32)
    codes = np.empty((N_IMG, L, K), np.float32)
    for core in range(N_CORES):
        sl = slice(core * IMG_PER_CORE, (core + 1) * IMG_PER_CORE)
        codes[sl] = (res.results[core]["codes"]
                     .reshape(T_PAD, K)[:T_TOK]
                     .reshape(IMG_PER_CORE, L, K))
        bow[sl] = res.results[core]["bow"].reshape(IMG_PER_CORE, K)
    return bow, codes


# revision 36
# speedup vs baseline: 1.0529x; 1.0077x over previous
"""BoW extractor (VQ codebook softmax + border-cropped mean pool) on 8 Trainium2 cores.

Data-parallel over the batch dim: each of the 8 NeuronCores handles 16 images.
Per core, tokens are flattened to [3136, 768] (padded to 3200 = 25 tiles of 128)
and processed tile-by-tile:
  logits = 30 * (x @ emb.T) / ||x||   (fp32r matmuls, C contracted in 6 chunks)
  codes  = exp(logits) / row_sum      (ACT exp with per-partition scale + fused
                                       row-sums, DVE reciprocal + scale)
  bow    = L1norm(mean of kept codes) (PE matmul against a precomputed selector
                                       W, PSUM results DVE-accumulated in SBUF)
The first 4 tiles are processed codebook-block-outer so the PE consumes the
codebook in DMA-arrival order (keeps the PE dense at startup, which also
keeps the HAM clock-gate open). ACT sqrt for the token norms is batched per
5 tiles (one pipeline stage ahead) so the exp/sqrt table sets are not
reloaded per tile. Measured ~403-405us on core 0, rel err ~4.3e-4 vs the
fp32 reference (fp32r matmul rounding).
"""
import sys

sys.path.insert(0, "/opt/trn_rl_repo")

import numpy as np

N_CORES = 8
N_IMG = 128
C = 768
K = 4096
L = 196  # tokens per image after dropping CLS
IMG_PER_CORE = N_IMG // N_CORES  # 16
T_TOK = IMG_PER_CORE * L  # 3136
NT = 25  # 128-token tiles per core
T_PAD = NT * 128  # 3200
GRID = 14
SKIP = 2
INV_DELTA = 30.0  # 15.0 / 0.5
NORMALIZE_EPS = 1e-5
NB = 5  # tiles per norm (sqrt) batch
NPRE = 5  # token tiles prefetched ahead of the codebook load

PROFILE = False
LAST_EXEC_NS = None

_PROG = None
_HOST_CONST = None


def _build_program():
    import concourse.bacc as bacc
    import concourse.tile as tile
    from concourse import mybir

    f32 = mybir.dt.float32
    f32r = mybir.dt.float32r
    bf16 = mybir.dt.bfloat16
    AF = mybir.ActivationFunctionType
    AX = mybir.AxisListType

    nc = bacc.Bacc("TRN2", target_bir_lowering=False, debug=False,
                   num_devices=N_CORES)
    xT_d = nc.dram_tensor("xT", [NT, 128, 6, 128], f32r, kind="ExternalInput")
    xn_d = nc.dram_tensor("xnat", [NT, 128, C], f32, kind="ExternalInput")
    emb_d = nc.dram_tensor("embT", [8, 128, 6, 512], f32r,
                           kind="ExternalInput")
    w_d = nc.dram_tensor("W", [128, NT, 16], f32r, kind="ExternalInput")
    codes_d = nc.dram_tensor("codes", [NT, 128, 8, 512], f32r,
                             kind="ExternalOutput")
    bow_d = nc.dram_tensor("bow", [16, 8, 512], f32, kind="ExternalOutput")

    with tile.TileContext(nc) as tc:
        with (
            tc.tile_pool(name="const", bufs=1) as constp,
            tc.tile_pool(name="xT", bufs=NPRE, space="SBUF") as xTp,
            tc.tile_pool(name="xn", bufs=2) as xnp_,
            tc.tile_pool(name="expp", bufs=4) as expp,
            tc.tile_pool(name="small", bufs=4) as smallp,
            tc.tile_pool(name="ps", bufs=6, space="PSUM") as psp,
            tc.tile_pool(name="psbow", bufs=2, space="PSUM") as psbowp,
        ):
            # --- PE warmup: dependency-free dummy matmuls run during the
            # initial DMAs so the HAM clock-gate opens before real work.
            warm_sb = constp.tile([128, 128], bf16)
            warm_ps = psp.tile([128, 512], f32, name="warm_ps", tag="ps")
            nc.vector.memset(warm_sb[:], 0.0)
            for i in range(60):
                nc.tensor.matmul(warm_ps[:, :128], warm_sb[:], warm_sb[:],
                                 start=True, stop=True)

            # --- prefetch the first NPRE token tiles ahead of the codebook
            # (xn batch 0 is sandwiched after the first two xT tiles: the
            # softmax scales gate PSUM evacuation, so the norm inputs must
            # land before the startup blocks fill all PSUM banks)
            pre_xT = []

            def prefetch_xT(t):
                a = xTp.tile([128, 6, 128], f32r, name=f"xTpre{t}",
                             tag="xT_t")
                nc.sync.dma_start(out=a[:], in_=xT_d.ap()[t])
                pre_xT.append(a)

            for t in range(2):
                prefetch_xT(t)

            bow_acc = constp.tile([16, 8, 512], f32)
            nc.gpsimd.memset(bow_acc[:], 0.0)

            # --- norm pipeline, one NB-tile batch ahead of the main loop:
            # squares accumulate ||x||^2, one batched ACT sqrt per NB tiles
            # (2 table-set reloads per batch instead of per tile)
            n2_all = constp.tile([128, NT], f32)
            sinv_all = constp.tile([128, NT], f32)

            def norm_batch(g):
                lo, hi = NB * g, min(NB * (g + 1), NT)
                for u in range(lo, hi):
                    xn_u = xnp_.tile([128, C], f32, name=f"xn{u}", tag="xn")
                    nc.sync.dma_start(out=xn_u[:], in_=xn_d.ap()[u])
                    nc.vector.scalar_tensor_tensor(
                        xn_u[:], xn_u[:], 1.0, xn_u[:],
                        mybir.AluOpType.mult, mybir.AluOpType.mult,
                        accum_out=n2_all[:, u:u + 1])
                nc.vector.tensor_scalar_max(n2_all[:, lo:hi],
                                            n2_all[:, lo:hi],
                                            NORMALIZE_EPS * NORMALIZE_EPS)
                nrm = smallp.tile([128, NB], f32, name=f"nrm{g}", tag="nrm")
                nc.scalar.activation(nrm[:, :hi - lo], n2_all[:, lo:hi],
                                     AF.Sqrt,
                                     scale=1.0 / (INV_DELTA * INV_DELTA))
                nc.vector.reciprocal(sinv_all[:, lo:hi], nrm[:, :hi - lo])

            norm_batch(0)
            for t in range(2, NPRE):
                prefetch_xT(t)

            # --- codebook load split across both HWDGE rings, interleaved
            # so blocks land roughly in consumption order
            emb_sb = constp.tile([128, 8, 6, 512], f32r)
            for j in range(8):
                # ACT ring serves the first-consumed blocks; the SP ring
                # (busy with x prefetches first) serves the later ones
                eng = nc.scalar if j < 3 else nc.sync
                eng.dma_start(out=emb_sb[:, j], in_=emb_d.ap()[j])
            w_sb = constp.tile([128, NT, 16], f32r)
            nc.scalar.dma_start(out=w_sb[:], in_=w_d.ap())

            def get_xT(t):
                if t < NPRE:
                    return pre_xT[t]
                xT_t = xTp.tile([128, 6, 128], f32r, name=f"xT{t}",
                                tag="xT_t")
                nc.sync.dma_start(out=xT_t[:], in_=xT_d.ap()[t])
                return xT_t

            def new_exp_sums(t):
                exp_t = expp.tile([128, 8, 512], f32r, name=f"exp{t}",
                                  tag="exp_t")
                sums = smallp.tile([128, 8], f32, name=f"sums{t}",
                                   tag="sums")
                return exp_t, sums

            def do_block(t, j, xT_t, exp_t, sums):
                ps = psp.tile([128, 512], f32, name=f"ps{t}_{j}", tag="ps")
                for c in range(6):
                    nc.tensor.matmul(ps[:], xT_t[:, c, :],
                                     emb_sb[:, j, c, :],
                                     start=(c == 0), stop=(c == 5))
                nc.scalar.activation(exp_t[:, j, :], ps[:], AF.Exp,
                                     scale=sinv_all[:, t:t + 1],
                                     accum_out=sums[:, j:j + 1])

            def do_post(t, exp_t, sums):
                # tile NT-1 holds only 64 real tokens (rest is padding)
                rows = 64 if t == NT - 1 else 128
                denom = smallp.tile([128, 1], f32, name=f"dn{t}",
                                    tag="denom")
                nc.vector.reduce_sum(denom[:rows], sums[:rows], axis=AX.X)
                r = smallp.tile([128, 1], f32, name=f"r{t}", tag="r")
                nc.vector.reciprocal(r[:rows], denom[:rows])
                for j in range(8):
                    nc.vector.tensor_scalar_mul(exp_t[:rows, j, :],
                                                exp_t[:rows, j, :], r[:rows])
                    bow_tmp = psbowp.tile([16, 512], f32,
                                          name=f"bt{t}_{j}", tag="bt")
                    nc.tensor.matmul(bow_tmp[:], w_sb[:rows, t, :],
                                     exp_t[:rows, j, :], start=True,
                                     stop=True)
                    nc.vector.tensor_add(bow_acc[:, j, :], bow_acc[:, j, :],
                                         bow_tmp[:])
                    if j % 2 == 1:
                        deng = nc.sync if t == NT - 1 else nc.gpsimd
                        deng.dma_start(
                            out=codes_d.ap()[t][:rows, j - 1:j + 1, :],
                            in_=exp_t[:rows, j - 1:j + 1, :])

            # --- startup: tiles 0-2 block-outer (follows emb DMA arrival)
            NSTART = 4
            start_bufs = [new_exp_sums(t) for t in range(NSTART)]
            for j in range(8):
                for t in range(NSTART):
                    do_block(t, j, pre_xT[t], *start_bufs[t])
            for t in range(NSTART):
                do_post(t, *start_bufs[t])

            # --- steady state
            batches_done = 1
            last_batch = (NT - 1) // NB
            for t in range(NSTART, NT):
                # keep the norm pipeline ~2 tiles ahead of consumption
                while batches_done <= min((t + 2) // NB, last_batch):
                    norm_batch(batches_done)
                    batches_done += 1
                xT_t = get_xT(t)
                exp_t, sums = new_exp_sums(t)
                for j in range(8):
                    do_block(t, j, xT_t, exp_t, sums)
                do_post(t, exp_t, sums)

            # --- L1-normalize bow per image (rows are images). Per-j
            # pipelining: ACT row-sums ride behind the final adds, the
            # scales alternate DVE/ACT, and each 32KB slice streams out on
            # the SP ring as it completes — the 16-partition bow tile is
            # port-starved for DMA, so starting transfers early matters.
            ssum_p = smallp.tile([16, 8], f32)
            for j in range(8):
                nc.scalar.activation(bow_acc[:, j, :], bow_acc[:, j, :],
                                     AF.Identity,
                                     accum_out=ssum_p[:, j:j + 1])
            ssum = smallp.tile([16, 1], f32)
            nc.vector.reduce_sum(ssum[:], ssum_p[:], axis=AX.X)
            nc.vector.tensor_scalar_max(ssum[:], ssum[:], NORMALIZE_EPS)
            rimg = smallp.tile([16, 1], f32)
            nc.vector.reciprocal(rimg[:], ssum[:])
            for j in range(8):
                if j % 2 == 0:
                    nc.vector.tensor_scalar_mul(bow_acc[:, j, :],
                                                bow_acc[:, j, :], rimg[:])
                else:
                    nc.scalar.activation(bow_acc[:, j, :],
                                         bow_acc[:, j, :], AF.Copy,
                                         scale=rimg[:])
                nc.sync.dma_start(out=bow_d.ap()[:, j, :],
                                  in_=bow_acc[:, j, :])

    nc.compile()
    return nc


def _host_constants():
    global _HOST_CONST
    if _HOST_CONST is not None:
        return _HOST_CONST
    # kept-token mask on the 14x14 grid (drop SKIP-wide border)
    l_idx = np.arange(L)
    row, col = l_idx // GRID, l_idx % GRID
    kept = ((row >= SKIP) & (row < GRID - SKIP) &
            (col >= SKIP) & (col < GRID - SKIP))
    n_keep = int(kept.sum())  # 100
    w_full = np.zeros((T_PAD, IMG_PER_CORE), np.float32)
    for i in range(IMG_PER_CORE):
        w_full[i * L:(i + 1) * L, i] = kept / float(n_keep)
    w3 = np.ascontiguousarray(
        w_full.reshape(NT, 128, IMG_PER_CORE).transpose(1, 0, 2))
    _HOST_CONST = w3
    return _HOST_CONST


def _get_program():
    global _PROG
    if _PROG is None:
        _PROG = _build_program()
    return _PROG


def kernel(x, embedding):
    global LAST_EXEC_NS
    from concourse.bass_utils import run_bass_kernel_spmd

    x = np.asarray(x, dtype=np.float32)
    embedding = np.asarray(embedding, dtype=np.float32)
    nc = _get_program()
    w3 = _host_constants()

    embT = np.ascontiguousarray(
        embedding.T.reshape(6, 128, 8, 512).transpose(2, 1, 0, 3))

    in_maps = []
    for core in range(N_CORES):
        xc = x[core * IMG_PER_CORE:(core + 1) * IMG_PER_CORE, 1:, :]
        xp = np.zeros((T_PAD, C), np.float32)
        xp[:T_TOK] = xc.reshape(T_TOK, C)
        xp[T_TOK:, 0] = 1.0  # pad tokens: unit norm, zero pool weight
        in_maps.append({
            "xT": np.ascontiguousarray(
                xp.reshape(NT, 128, 6, 128).transpose(0, 3, 2, 1)),
            "xnat": xp.reshape(NT, 128, C),
            "embT": embT,
            "W": w3,
        })

    res = run_bass_kernel_spmd(nc, in_maps, core_ids=list(range(N_CORES)),
                               trace=PROFILE)
    LAST_EXEC_NS = res.exec_time_ns

    bow = np.empty((N_IMG, K), np.float32)
    codes = np.empty((N_IMG, L, K), np.float32)
    for core in range(N_CORES):
        sl = slice(core * IMG_PER_CORE, (core + 1) * IMG_PER_CORE)
        codes[sl] = (res.results[core]["codes"]
                     .reshape(T_PAD, K)[:T_TOK]
                     .reshape(IMG_PER_CORE, L, K))
        bow[sl] = res.results[core]["bow"].reshape(IMG_PER_CORE, K)
    return bow, codes
